# revision 37
# baseline (speedup 1.0000x reference)
"""Trainium2 Bass kernel for batched multi-head attention (B=2, S=2048, E=1024, H=16).

Sharding: core r = 4*b + g handles batch b and head-group g (4 heads, 256 emb cols).
- QKV projections: tensor-parallel over head groups (each core computes its 256
  output cols from the full 1024-dim input, streamed in 128-row blocks).
- Attention: each core runs 4 heads over all 2048 queries (scores kept transposed
  [kt, qt]; softmax normalization deferred via a mask/ones column appended to V).
- Out-proj: token-parallel. Contexts are exchanged with a single 8-rank AllToAll
  per head (4-rank groups unsupported by the mesh collective, so cross-batch
  slots are pre-zeroed once via DMA and the receiver sums slot pairs j and j+4).
  Each core then computes its 512-token slice of the output.
Attention + projections + out-proj all run in bf16 (1 cycle/col on the PE vs 2
for f32r); PSUM accumulation is fp32 throughout, final output is fp32.
"""

import sys

if '/opt/trn_rl_repo' not in sys.path:
    sys.path.insert(0, '/opt/trn_rl_repo')

import numpy as np

P = 128
B, S, E, H, DH = 2, 2048, 1024, 16, 64
NCORES = 8
G = 4                 # head groups == cores per batch
EG = E // G           # 256 emb cols per group
TS = S // G           # 512 tokens per core in out-proj
KB = S // P           # 16 key-token blocks
IB = E // P           # 8 contraction blocks of 128
QW = 512              # matmul moving free-dim chunk
SCALE = DH ** -0.5
DEBUG = False          # adds intermediate-dump outputs (work/dbg_check.py)
# bit-trick exp -> fp8e4m3 on the DVE for half the heads (offloads the
# Scalar engine): fp8 bits of e^(s*SCALE) ~= round(EA*s + EB)
EA = 8 * 1.4426950408889634 * SCALE
EB = 55.5

_cache = {}


def _build():
    import concourse.bass as bass
    import concourse.mybir as mybir
    import concourse.tile as tile
    from concourse import bacc
    from contextlib import ExitStack

    f32 = mybir.dt.float32
    bf16 = mybir.dt.bfloat16
    fp8 = mybir.dt.float8e4
    AF = mybir.ActivationFunctionType
    DR = mybir.MatmulPerfMode.DoubleRow

    nc = bacc.Bacc("TRN2", target_bir_lowering=False, debug=False,
                   num_devices=NCORES)

    xqT = nc.dram_tensor("xqT", [E, S], bf16, kind="ExternalInput").ap()
    xkT = nc.dram_tensor("xkT", [E, S], bf16, kind="ExternalInput").ap()
    xvT = nc.dram_tensor("xvT", [E, S], bf16, kind="ExternalInput").ap()
    wqT = nc.dram_tensor("wqT", [E, EG], bf16, kind="ExternalInput").ap()
    wkT = nc.dram_tensor("wkT", [E, EG], bf16, kind="ExternalInput").ap()
    wvT = nc.dram_tensor("wvT", [E, EG], bf16, kind="ExternalInput").ap()
    woT = nc.dram_tensor("woT", [E, E], bf16, kind="ExternalInput").ap()
    # per-partition bias columns: q_m0, q_m1, k_m0, k_m1, v_m0, v_m1
    bcols = nc.dram_tensor("bcols", [P, 6], f32, kind="ExternalInput").ap()
    # per-rank A2A slot masks [P, 2]: col 0 = keep low slots (batch 0),
    # col 1 = keep high slots (batch 1)
    zcols = nc.dram_tensor("zcols", [P, 2], f32, kind="ExternalInput").ap()
    bov = nc.dram_tensor("bov", [1, E], bf16, kind="ExternalInput").ap()
    onesv = nc.dram_tensor("onesv", [1, P], bf16, kind="ExternalInput").ap()
    mask_pb = nc.dram_tensor("mask_pb", [P, KB], f32, kind="ExternalInput").ap()
    maskrep = nc.dram_tensor("maskrep", [P, KB * G], f32, kind="ExternalInput").ap()
    out = nc.dram_tensor("out", [TS, E], f32, kind="ExternalOutput").ap()
    if DEBUG:
        dbg_qp = nc.dram_tensor("dbg_qp", [P, S], bf16, kind="ExternalOutput").ap()
        dbg_kp = nc.dram_tensor("dbg_kp", [P, S], bf16, kind="ExternalOutput").ap()
        dbg_vp = nc.dram_tensor("dbg_vp", [P, 2 * G * 80], fp8,
                                kind="ExternalOutput").ap()
        dbg_ctx = nc.dram_tensor("dbg_ctx", [DH, S], bf16,
                                 kind="ExternalOutput").ap()

    # one exchange per head pair: slot s carries both heads' contexts
    a2a_ins = [nc.dram_tensor(f"a2a_in{p}", [NCORES, 2, DH, TS], bf16).ap()
               for p in range(2)]
    a2a_outs = [nc.dram_tensor(f"a2a_out{p}", [NCORES, 2, DH, TS], bf16).ap()
                for p in range(2)]

    with tile.TileContext(nc) as tc, ExitStack() as top:
        const = top.enter_context(tc.tile_pool(name="const", bufs=1))

        ones_r = const.tile([1, P], bf16)
        nc.sync.dma_start(ones_r[:], onesv[:])
        bcol_t = const.tile([P, 6], f32)
        nc.sync.dma_start(bcol_t[:], bcols[:])
        bo_r = const.tile([1, E], bf16)
        nc.sync.dma_start(bo_r[:], bov[:])
        mask_t = const.tile([P, KB], f32)
        nc.sync.dma_start(mask_t[:], mask_pb[:])
        maskrep_t = const.tile([P, KB * G], f32)
        nc.sync.dma_start(maskrep_t[:], maskrep[:])
        zcol_t = const.tile([P, 2], f32)
        nc.sync.dma_start(zcol_t[:], zcols[:])

        # persistent projection outputs
        VPAD = 80  # per-head stride in vp_dr (>= DH+1, byte-step % 16 == 0)
        proj_sb = top.enter_context(tc.tile_pool(name="proj_sb", bufs=1))
        qpT = [proj_sb.tile([P, S], bf16, tag=f"qpT{m}", name=f"qpT{m}")
               for m in range(2)]
        kpT = [proj_sb.tile([P, S], bf16, tag=f"kpT{m}", name=f"kpT{m}")
               for m in range(2)]
        # vp_dr: per j-pair, fp8 DoubleRow stationary [P, (jj 2) x (h 4) x VPAD]
        vp_dr = [proj_sb.tile([P, 2 * G * VPAD], fp8, tag=f"vpdr{m}",
                              name=f"vpdr{m}") for m in range(KB // 2)]

        # ---- projection weights (V first — its projection runs first) ----
        with tc.tile_pool(name="wqkv", bufs=1) as wqkv:
            w_r = {}
            for name, wap in (("v", wvT), ("k", wkT), ("q", wqT)):
                wr = wqkv.tile([P, IB * EG], bf16, tag=f"w{name}r", name=f"w{name}r")
                for i in range(IB):
                    nc.gpsimd.dma_start(wr[:, i * EG:(i + 1) * EG],
                                        wap[i * P:(i + 1) * P, :])
                w_r[name] = wr

            # ---- V, K and Q projections: out [e_sel, t] transposed ----
            with tc.tile_pool(name="vpT_p", bufs=1) as vpT_p:
              vpT = [vpT_p.tile([P, S], bf16, tag=f"vpT{m}", name=f"vpT{m}")
                     for m in range(2)]
              vp_tok = vpT_p.tile([P, KB * EG], bf16, tag="vp_tok", name="vp_tok")
              with tc.tile_pool(name="xst", bufs=8) as xst, \
                 tc.tile_pool(name="kqpsum", bufs=1, space="PSUM") as kqpsum:
                for bi, (name, xap, dsts) in enumerate((("v", xvT, vpT),
                                                        ("k", xkT, kpT),
                                                        ("q", xqT, qpT))):
                    kqs = [kqpsum.tile([P, S], f32, tag=f"kqs{m}", name=f"kqs{m}")
                           for m in range(2)]
                    for i in range(IB):
                        xr = xst.tile([P, S], bf16)
                        # alternate DMA issue engines to spread hw queues
                        eng = nc.sync if i % 2 == 0 else nc.scalar
                        eng.dma_start(xr[:], xap[i * P:(i + 1) * P, :])
                        for m in range(2):
                            for c in range(S // QW):
                                nc.tensor.matmul(
                                    kqs[m][:, c * QW:(c + 1) * QW],
                                    w_r[name][:, i * EG + m * P:i * EG + (m + 1) * P],
                                    xr[:, c * QW:(c + 1) * QW],
                                    start=(i == 0), stop=(i == IB - 1))
                    # bias col index: q=0, k=2, v=4 (+m)
                    bci = {"q": 0, "k": 2, "v": 4}[name]
                    for m in range(2):
                        nc.vector.tensor_scalar_add(
                            dsts[m][:], kqs[m][:], bcol_t[:, bci + m:bci + m + 1])
                    if name == "v":
                        # transpose V to token-major and build the fp8
                        # DoubleRow stationary tiles NOW, overlapping the
                        # K and Q projections (PE) with this DVE work.
                        SQ = 32
                        for m2 in range(2):
                            for a in range(P // SQ):
                                for b_ in range(P // SQ):
                                    dst = vp_tok[a * SQ:(a + 1) * SQ, :] \
                                        .rearrange("p (kt e) -> p kt e", e=EG)[
                                            :, :, m2 * P + b_ * SQ:m2 * P + (b_ + 1) * SQ]
                                    srcb = vpT[m2][b_ * SQ:(b_ + 1) * SQ, :] \
                                        .rearrange("p (kt t) -> p kt t", t=P)[
                                            :, :, a * SQ:(a + 1) * SQ]
                                    nc.vector.transpose(dst, srcb)
                        for m in range(KB):
                            jp, jj = m // 2, m % 2
                            src3 = vp_tok[:, m * EG:(m + 1) * EG] \
                                .rearrange("p (o h d) -> p o h d", o=1, h=G)
                            dst4 = vp_dr[jp] \
                                .rearrange("p (jj h e) -> p jj h e", jj=2, e=VPAD)
                            nc.vector.tensor_scalar_mul(
                                dst4[:, jj:jj + 1, :, 0:DH], src3,
                                mask_t[:, m:m + 1])
                            nc.vector.tensor_copy(
                                dst4[:, jj:jj + 1, :, DH:DH + 1],
                                maskrep_t[:, m * G:(m + 1) * G]
                                .rearrange("p (o h e) -> p o h e", o=1, e=1))

        if DEBUG:
            nc.sync.dma_start(dbg_qp[:], qpT[0][:])
            nc.sync.dma_start(dbg_kp[:], kpT[0][:])
            nc.sync.dma_start(dbg_vp[:], vp_dr[0][:])

        # ---- out-proj weights: loaded during attention ----
        wo_pool = top.enter_context(tc.tile_pool(name="wo", bufs=1))
        wo_r = wo_pool.tile([P, IB * E], bf16)
        for i in range(IB):
            nc.gpsimd.dma_start(wo_r[:, i * E:(i + 1) * E],
                                woT[i * P:(i + 1) * P, :])

        # gather tiles: built per pair as soon as its AllToAll lands
        gap = top.enter_context(tc.tile_pool(name="gap", bufs=1))
        gstage = top.enter_context(tc.tile_pool(name="gstage", bufs=4))
        ga = {}

        def build_ga(m):
            # m == pair index: heads (2m, 2m+1) land in rows 0/64 of ga
            for gp in range(G):
                ib = gp * 2 + m
                gt = gap.tile([P, TS], bf16, tag=f"ga{ib}", name=f"ga{ib}")
                t0 = gstage.tile([P, TS], bf16, tag="g0", name="g0")
                t1 = gstage.tile([P, TS], bf16, tag="g1", name="g1")
                for hl in range(2):
                    nc.gpsimd.dma_start(t0[hl * DH:(hl + 1) * DH, :],
                                        a2a_outs[m][gp, hl])
                    nc.gpsimd.dma_start(t1[hl * DH:(hl + 1) * DH, :],
                                        a2a_outs[m][gp + 4, hl])
                nc.vector.tensor_add(gt[:], t0[:], t1[:])
                ga[ib] = gt

        # ---- attention ----
        # Head pairs (2*pr, 2*pr+1) run together: their score matmuls have
        # K=DH=64 and sit at partition offsets 0/64 of kpT/qpT[pr], so they
        # row-tile onto disjoint halves of the PE array and run concurrently.
        # One exp ACTIVATE covers both heads' scores for a (q-chunk, j).
        # PV runs in fp8 DoubleRow: contraction over 2 key blocks per matmul.
        with tc.tile_pool(name="spsum", bufs=2, space="PSUM") as spsum, \
             tc.tile_pool(name="pvpsum", bufs=2, space="PSUM") as pvpsum, \
             tc.tile_pool(name="expp", bufs=4) as expp, \
             tc.tile_pool(name="normp", bufs=3) as normp:
            for pr in range(2):
                for qc in range(G):          # 512-token q chunks == out shards
                    q0 = qc * QW
                    pvA = pvpsum.tile([DH + 1, QW], f32, tag="pvA", name="pvA")
                    pvB = pvpsum.tile([DH + 1, QW], f32, tag="pvB", name="pvB")
                    for jp in range(KB // 2):
                        es = expp.tile([P, 4 * QW], fp8)  # [(jj 2) x (h 2) x QW]
                        for jj in range(2):
                            j = jp * 2 + jj
                            sp = spsum.tile([P, 2 * QW], f32)
                            nc.tensor.matmul(
                                sp[:, 0:QW],
                                kpT[pr][0:DH, j * P:(j + 1) * P],
                                qpT[pr][0:DH, q0:q0 + QW],
                                start=True, stop=True)
                            nc.tensor.matmul(
                                sp[:, QW:2 * QW],
                                kpT[pr][DH:P, j * P:(j + 1) * P],
                                qpT[pr][DH:P, q0:q0 + QW],
                                start=True, stop=True)
                            nc.scalar.activation(
                                es[:, jj * 2 * QW:(jj + 1) * 2 * QW], sp[:],
                                AF.Exp, scale=SCALE)
                        for hloc, pv_ in ((0, pvA), (1, pvB)):
                            lhs = vp_dr[jp].rearrange(
                                "p (jj h e) -> p jj h e", jj=2, e=VPAD)[
                                :, :, 2 * pr + hloc, 0:DH + 1]
                            rhs = es.rearrange("p (jj he) -> p jj he", jj=2)[
                                :, :, hloc * QW:(hloc + 1) * QW]
                            nc.tensor.matmul(
                                pv_[:], lhs, rhs,
                                start=(jp == 0), stop=(jp == KB // 2 - 1),
                                perf_mode=DR)
                    # normalize + send both heads' q-chunk
                    for hloc, pv_ in ((0, pvA), (1, pvB)):
                        h = 2 * pr + hloc
                        srow = normp.tile([1, QW], f32, tag="srow", name="srow")
                        nc.vector.tensor_copy(srow[:], pv_[DH:DH + 1, :])
                        rec = normp.tile([1, QW], f32, tag="rec", name="rec")
                        nc.vector.reciprocal_approx_fast(rec[:], srow[:])
                        recB = normp.tile([DH, QW], f32, tag="recB", name="recB")
                        nc.gpsimd.partition_broadcast(recB[:], rec[:])
                        ctxn = normp.tile([DH, QW], bf16, tag="ctxn", name="ctxn")
                        nc.vector.tensor_mul(ctxn[:], pv_[0:DH, :], recB[:])
                        if DEBUG and h == 0:
                            nc.sync.dma_start(dbg_ctx[:, q0:q0 + QW], ctxn[:])
                        # receiver sums slot pairs (j, j+4); zero the copy
                        # belonging to the other batch (zcol is 1/0 per rank)
                        ctx0 = normp.tile([DH, QW], bf16, tag="ctx0", name="ctx0")
                        nc.vector.tensor_scalar_mul(ctx0[:], ctxn[:],
                                                    zcol_t[0:DH, 0:1])
                        ctx1 = normp.tile([DH, QW], bf16, tag="ctx1", name="ctx1")
                        nc.vector.tensor_scalar_mul(ctx1[:], ctxn[:],
                                                    zcol_t[0:DH, 1:2])
                        nc.sync.dma_start(a2a_ins[pr][qc, hloc], ctx0[:])
                        nc.sync.dma_start(a2a_ins[pr][qc + 4, hloc], ctx1[:])
                    # gather pair 0's exchange once it has certainly landed;
                    # the wait hint keeps the scheduler from hoisting these
                    # sem-blocked DMAs ahead of this pair's normalize work
                    if pr == 1 and qc == 3:
                        with tc.tile_wait_until(0.25):
                            build_ga(0)
                # fire this pair's exchange once all q-chunks are sent
                nc.gpsimd.collective_compute(
                    "AllToAll", mybir.AluOpType.bypass,
                    replica_groups=[list(range(NCORES))],
                    ins=[a2a_ins[pr][:]], outs=[a2a_outs[pr][:]])

        # ---- out-proj on my 512-token slice ----
        # Two passes over ib groups: the m=0 half (heads 0/1, available after
        # pair 0's AllToAll) accumulates for ALL token tiles first, so the PE
        # stays busy while pair 1's AllToAll is still in flight; the m=1 half
        # + bias finishes each tile afterwards.
        with tc.tile_pool(name="opsum", bufs=1, space="PSUM") as opsum, \
             tc.tile_pool(name="outsb", bufs=2) as outsb:
            build_ga(1)
            pots = [opsum.tile([P, E], f32, tag=f"pot{tm}", name=f"pot{tm}")
                    for tm in range(TS // P)]
            for m in range(2):
                for tm in range(TS // P):
                    pot = pots[tm]
                    for n, gp in enumerate(range(G)):
                        ib = gp * 2 + m
                        for oc in range(E // QW):
                            nc.tensor.matmul(
                                pot[:, oc * QW:(oc + 1) * QW],
                                ga[ib][:, tm * P:(tm + 1) * P],
                                wo_r[:, ib * E + oc * QW:ib * E + oc * QW + QW],
                                start=(m == 0 and n == 0), stop=False)
                    if m == 1:
                        for oc in range(E // QW):
                            nc.tensor.matmul(
                                pot[:, oc * QW:(oc + 1) * QW],
                                ones_r[0:1, 0:P],
                                bo_r[0:1, oc * QW:(oc + 1) * QW],
                                start=False, stop=True)
                        ot = outsb.tile([P, E], f32)
                        nc.scalar.activation(ot[:], pot[:], AF.Copy)
                        nc.sync.dma_start(out[tm * P:(tm + 1) * P, :], ot[:])

    nc.compile()
    return nc


def _get_nc():
    if 'nc' not in _cache:
        _cache['nc'] = _build()
    return _cache['nc']


def kernel(q, k, v, mask, Wq, bq, Wk, bk, Wv, bv, Wo, bo):
    from concourse.bass_utils import run_bass_kernel_spmd
    import ml_dtypes

    nc = _get_nc()
    f32 = np.float32
    bft = ml_dtypes.bfloat16

    def _cvt(x):
        return np.ascontiguousarray(np.asarray(x, f32)).astype(bft)

    qT = [_cvt(np.asarray(q, f32)[b].T) for b in range(B)]
    kT = [_cvt(np.asarray(k, f32)[b].T) for b in range(B)]
    vT = [_cvt(np.asarray(v, f32)[b].T) for b in range(B)]
    WqT = _cvt(np.asarray(Wq, f32).T)
    WkT = _cvt(np.asarray(Wk, f32).T)
    WvT = _cvt(np.asarray(Wv, f32).T)
    WoT = _cvt(np.asarray(Wo, f32).T)
    bqf = np.asarray(bq, f32)
    bkf = np.asarray(bk, f32)
    bvf = np.asarray(bv, f32)
    bof = _cvt(np.asarray(bo, f32))
    onesr = np.ones((1, P), bft)
    maskf = (np.asarray(mask) != 0).astype(f32)  # [B, S]

    in_maps = []
    for r in range(NCORES):
        b, g = r // G, r % G
        cols = slice(g * EG, (g + 1) * EG)
        m_pb = np.ascontiguousarray(maskf[b].reshape(KB, P).T)       # [128,16]
        m_rep = np.ascontiguousarray(np.repeat(m_pb, G, axis=1))     # [128,64]
        # bias columns [P, 6]: q_m0, q_m1, k_m0, k_m1, v_m0, v_m1
        bc = np.stack([bqf[cols].reshape(2, P)[0], bqf[cols].reshape(2, P)[1],
                       bkf[cols].reshape(2, P)[0], bkf[cols].reshape(2, P)[1],
                       bvf[cols].reshape(2, P)[0], bvf[cols].reshape(2, P)[1]],
                      axis=1).astype(f32)
        zc = np.zeros((P, 2), f32)
        zc[:, b] = 1.0
        in_maps.append({
            "xqT": qT[b], "xkT": kT[b], "xvT": vT[b],
            "wqT": np.ascontiguousarray(WqT[:, cols]),
            "wkT": np.ascontiguousarray(WkT[:, cols]),
            "wvT": np.ascontiguousarray(WvT[:, cols]),
            "woT": WoT,
            "bcols": np.ascontiguousarray(bc),
            "zcols": zc,
            "bov": bof[None, :],
            "onesv": onesr,
            "mask_pb": m_pb, "maskrep": m_rep,
        })

    res = run_bass_kernel_spmd(nc, in_maps, core_ids=list(range(NCORES)),
                               **_cache.get('run_kwargs', {}))
    _cache['last_results'] = res

    full = np.empty((B, S, E), f32)
    for r in range(NCORES):
        b, g = r // G, r % G
        full[b, g * TS:(g + 1) * TS, :] = res.results[r]["out"]
    return full


# revision 38
# speedup vs baseline: 1.1948x; 1.1948x over previous
"""Trainium2 Bass kernel for batched multi-head attention (B=2, S=2048, E=1024, H=16).

Sharding: core r = 4*b + g handles batch b and head-group g (4 heads, 256 emb cols).
- QKV projections: tensor-parallel over head groups (each core computes its 256
  output cols from the full 1024-dim input, streamed in 128-row blocks).
- Attention: each core runs 4 heads over all 2048 queries (scores kept transposed
  [kt, qt]; softmax normalization deferred via a mask/ones column appended to V).
- Out-proj: token-parallel. Contexts are exchanged with a single 8-rank AllToAll
  per head (4-rank groups unsupported by the mesh collective, so cross-batch
  slots are pre-zeroed once via DMA and the receiver sums slot pairs j and j+4).
  Each core then computes its 512-token slice of the output.
Attention + projections + out-proj all run in bf16 (1 cycle/col on the PE vs 2
for f32r); PSUM accumulation is fp32 throughout, final output is fp32.
"""

import sys

if '/opt/trn_rl_repo' not in sys.path:
    sys.path.insert(0, '/opt/trn_rl_repo')

import numpy as np

P = 128
B, S, E, H, DH = 2, 2048, 1024, 16, 64
NCORES = 8
G = 4                 # head groups == cores per batch
EG = E // G           # 256 emb cols per group
TS = S // G           # 512 tokens per core in out-proj
KB = S // P           # 16 key-token blocks
IB = E // P           # 8 contraction blocks of 128
QW = 512              # matmul moving free-dim chunk
SCALE = DH ** -0.5
DEBUG = False          # adds intermediate-dump outputs (work/dbg_check.py)
# bit-trick exp -> fp8e4m3 on the DVE for half the heads (offloads the
# Scalar engine): fp8 bits of e^(s*SCALE) ~= round(EA*s + EB)
EA = 8 * 1.4426950408889634 * SCALE
EB = 55.5

_cache = {}


def _build():
    import concourse.bass as bass
    import concourse.mybir as mybir
    import concourse.tile as tile
    from concourse import bacc
    from contextlib import ExitStack

    f32 = mybir.dt.float32
    bf16 = mybir.dt.bfloat16
    fp8 = mybir.dt.float8e4
    AF = mybir.ActivationFunctionType
    DR = mybir.MatmulPerfMode.DoubleRow

    nc = bacc.Bacc("TRN2", target_bir_lowering=False, debug=False,
                   num_devices=NCORES)

    xqT = nc.dram_tensor("xqT", [E, S], bf16, kind="ExternalInput").ap()
    xkT = nc.dram_tensor("xkT", [E, S], bf16, kind="ExternalInput").ap()
    xvT = nc.dram_tensor("xvT", [E, S], bf16, kind="ExternalInput").ap()
    wqT = nc.dram_tensor("wqT", [E, EG], bf16, kind="ExternalInput").ap()
    wkT = nc.dram_tensor("wkT", [E, EG], bf16, kind="ExternalInput").ap()
    wvT = nc.dram_tensor("wvT", [E, EG], bf16, kind="ExternalInput").ap()
    woT = nc.dram_tensor("woT", [E, E], bf16, kind="ExternalInput").ap()
    # per-partition bias columns: q_m0, q_m1, k_m0, k_m1, v_m0, v_m1
    bcols = nc.dram_tensor("bcols", [P, 6], f32, kind="ExternalInput").ap()
    # per-rank A2A slot masks [P, 2]: col 0 = keep low slots (batch 0),
    # col 1 = keep high slots (batch 1)
    zcols = nc.dram_tensor("zcols", [P, 2], f32, kind="ExternalInput").ap()
    bov = nc.dram_tensor("bov", [1, E], bf16, kind="ExternalInput").ap()
    onesv = nc.dram_tensor("onesv", [1, P], bf16, kind="ExternalInput").ap()
    mask_pb = nc.dram_tensor("mask_pb", [P, KB], f32, kind="ExternalInput").ap()
    maskrep = nc.dram_tensor("maskrep", [P, KB * G], f32, kind="ExternalInput").ap()
    out = nc.dram_tensor("out", [TS, E], f32, kind="ExternalOutput").ap()
    if DEBUG:
        dbg_qp = nc.dram_tensor("dbg_qp", [P, S], bf16, kind="ExternalOutput").ap()
        dbg_kp = nc.dram_tensor("dbg_kp", [P, S], bf16, kind="ExternalOutput").ap()
        dbg_vp = nc.dram_tensor("dbg_vp", [P, 2 * G * 80], fp8,
                                kind="ExternalOutput").ap()
        dbg_ctx = nc.dram_tensor("dbg_ctx", [DH, S], bf16,
                                 kind="ExternalOutput").ap()

    # one exchange per head pair: slot s carries both heads' contexts
    a2a_ins = [nc.dram_tensor(f"a2a_in{p}", [NCORES, 2, DH, TS], bf16).ap()
               for p in range(2)]
    a2a_outs = [nc.dram_tensor(f"a2a_out{p}", [NCORES, 2, DH, TS], bf16).ap()
                for p in range(2)]

    with tile.TileContext(nc) as tc, ExitStack() as top:
        const = top.enter_context(tc.tile_pool(name="const", bufs=1))

        ones_r = const.tile([1, P], bf16)
        nc.sync.dma_start(ones_r[:], onesv[:])
        bcol_t = const.tile([P, 6], f32)
        nc.sync.dma_start(bcol_t[:], bcols[:])
        bo_r = const.tile([1, E], bf16)
        nc.sync.dma_start(bo_r[:], bov[:])
        mask_t = const.tile([P, KB], f32)
        nc.sync.dma_start(mask_t[:], mask_pb[:])
        maskrep_t = const.tile([P, KB * G], f32)
        nc.sync.dma_start(maskrep_t[:], maskrep[:])
        zcol_t = const.tile([P, 2], f32)
        nc.sync.dma_start(zcol_t[:], zcols[:])

        # persistent projection outputs
        VPAD = 80  # per-head stride in vp_dr (>= DH+1, byte-step % 16 == 0)
        proj_sb = top.enter_context(tc.tile_pool(name="proj_sb", bufs=1))
        qpT = [proj_sb.tile([P, S], bf16, tag=f"qpT{m}", name=f"qpT{m}")
               for m in range(2)]
        kpT = [proj_sb.tile([P, S], bf16, tag=f"kpT{m}", name=f"kpT{m}")
               for m in range(2)]
        # vp_dr: per j-pair, fp8 DoubleRow stationary [P, (jj 2) x (h 4) x VPAD]
        vp_dr = [proj_sb.tile([P, 2 * G * VPAD], fp8, tag=f"vpdr{m}",
                              name=f"vpdr{m}") for m in range(KB // 2)]

        # ---- projection weights (V first — its projection runs first) ----
        with tc.tile_pool(name="wqkv", bufs=1) as wqkv:
            w_r = {}
            for name, wap in (("v", wvT), ("k", wkT), ("q", wqT)):
                wr = wqkv.tile([P, IB * EG], bf16, tag=f"w{name}r", name=f"w{name}r")
                for i in range(IB):
                    nc.gpsimd.dma_start(wr[:, i * EG:(i + 1) * EG],
                                        wap[i * P:(i + 1) * P, :])
                w_r[name] = wr

            # ---- V, K and Q projections: out [e_sel, t] transposed ----
            with tc.tile_pool(name="vpT_p", bufs=1) as vpT_p:
              vpT = [vpT_p.tile([P, S], bf16, tag=f"vpT{m}", name=f"vpT{m}")
                     for m in range(2)]
              vp_tok = vpT_p.tile([P, KB * EG], bf16, tag="vp_tok", name="vp_tok")
              with tc.tile_pool(name="xst", bufs=8) as xst, \
                 tc.tile_pool(name="kqpsum", bufs=1, space="PSUM") as kqpsum:
                for bi, (name, xap, dsts) in enumerate((("v", xvT, vpT),
                                                        ("k", xkT, kpT),
                                                        ("q", xqT, qpT))):
                    kqs = [kqpsum.tile([P, S], f32, tag=f"kqs{m}", name=f"kqs{m}")
                           for m in range(2)]
                    for i in range(IB):
                        xr = xst.tile([P, S], bf16)
                        # alternate DMA issue engines to spread hw queues
                        eng = nc.sync if i % 2 == 0 else nc.scalar
                        eng.dma_start(xr[:], xap[i * P:(i + 1) * P, :])
                        for m in range(2):
                            for c in range(S // QW):
                                nc.tensor.matmul(
                                    kqs[m][:, c * QW:(c + 1) * QW],
                                    w_r[name][:, i * EG + m * P:i * EG + (m + 1) * P],
                                    xr[:, c * QW:(c + 1) * QW],
                                    start=(i == 0), stop=(i == IB - 1))
                    # bias col index: q=0, k=2, v=4 (+m)
                    bci = {"q": 0, "k": 2, "v": 4}[name]
                    for m in range(2):
                        nc.vector.tensor_scalar_add(
                            dsts[m][:], kqs[m][:], bcol_t[:, bci + m:bci + m + 1])
                    if name == "v":
                        # transpose V to token-major and build the fp8
                        # DoubleRow stationary tiles NOW, overlapping the
                        # K and Q projections (PE) with this DVE work.
                        SQ = 32
                        for m2 in range(2):
                            for a in range(P // SQ):
                                for b_ in range(P // SQ):
                                    dst = vp_tok[a * SQ:(a + 1) * SQ, :] \
                                        .rearrange("p (kt e) -> p kt e", e=EG)[
                                            :, :, m2 * P + b_ * SQ:m2 * P + (b_ + 1) * SQ]
                                    srcb = vpT[m2][b_ * SQ:(b_ + 1) * SQ, :] \
                                        .rearrange("p (kt t) -> p kt t", t=P)[
                                            :, :, a * SQ:(a + 1) * SQ]
                                    nc.vector.transpose(dst, srcb)
                        for m in range(KB):
                            jp, jj = m // 2, m % 2
                            src3 = vp_tok[:, m * EG:(m + 1) * EG] \
                                .rearrange("p (o h d) -> p o h d", o=1, h=G)
                            dst4 = vp_dr[jp] \
                                .rearrange("p (jj h e) -> p jj h e", jj=2, e=VPAD)
                            nc.vector.tensor_scalar_mul(
                                dst4[:, jj:jj + 1, :, 0:DH], src3,
                                mask_t[:, m:m + 1])
                            nc.vector.tensor_copy(
                                dst4[:, jj:jj + 1, :, DH:DH + 1],
                                maskrep_t[:, m * G:(m + 1) * G]
                                .rearrange("p (o h e) -> p o h e", o=1, e=1))

        if DEBUG:
            nc.sync.dma_start(dbg_qp[:], qpT[0][:])
            nc.sync.dma_start(dbg_kp[:], kpT[0][:])
            nc.sync.dma_start(dbg_vp[:], vp_dr[0][:])

        # ---- out-proj weights: loaded during attention ----
        wo_pool = top.enter_context(tc.tile_pool(name="wo", bufs=1))
        wo_r = wo_pool.tile([P, IB * E], bf16)
        for i in range(IB):
            nc.gpsimd.dma_start(wo_r[:, i * E:(i + 1) * E],
                                woT[i * P:(i + 1) * P, :])

        # gather tiles: built per pair as soon as its AllToAll lands
        gap = top.enter_context(tc.tile_pool(name="gap", bufs=1))
        gstage = top.enter_context(tc.tile_pool(name="gstage", bufs=4))
        ga = {}

        def build_ga(m):
            # m == pair index: heads (2m, 2m+1) land in rows 0/64 of ga
            for gp in range(G):
                ib = gp * 2 + m
                gt = gap.tile([P, TS], bf16, tag=f"ga{ib}", name=f"ga{ib}")
                t0 = gstage.tile([P, TS], bf16, tag="g0", name="g0")
                t1 = gstage.tile([P, TS], bf16, tag="g1", name="g1")
                for hl in range(2):
                    nc.gpsimd.dma_start(t0[hl * DH:(hl + 1) * DH, :],
                                        a2a_outs[m][gp, hl])
                    nc.gpsimd.dma_start(t1[hl * DH:(hl + 1) * DH, :],
                                        a2a_outs[m][gp + 4, hl])
                nc.vector.tensor_add(gt[:], t0[:], t1[:])
                ga[ib] = gt

        # ---- attention ----
        # Head pairs (2*pr, 2*pr+1) run together: their score matmuls have
        # K=DH=64 and sit at partition offsets 0/64 of kpT/qpT[pr], so they
        # row-tile onto disjoint halves of the PE array and run concurrently.
        # One exp ACTIVATE covers both heads' scores for a (q-chunk, j).
        # PV runs in fp8 DoubleRow: contraction over 2 key blocks per matmul.
        with tc.tile_pool(name="spsum", bufs=3, space="PSUM") as spsum, \
             tc.tile_pool(name="pvpsum", bufs=1, space="PSUM") as pvpsum, \
             tc.tile_pool(name="expp", bufs=4) as expp, \
             tc.tile_pool(name="normp", bufs=3) as normp:
            for pr in range(2):
                for qc in range(G):          # 512-token q chunks == out shards
                    q0 = qc * QW
                    pvA = pvpsum.tile([DH + 1, QW], f32, tag="pvA", name="pvA")
                    pvB = pvpsum.tile([DH + 1, QW], f32, tag="pvB", name="pvB")
                    for jp in range(KB // 2):
                        es = expp.tile([P, 4 * QW], fp8)  # [(jj 2) x (h 2) x QW]
                        for jj in range(2):
                            j = jp * 2 + jj
                            sp = spsum.tile([P, 2 * QW], f32)
                            nc.tensor.matmul(
                                sp[:, 0:QW],
                                kpT[pr][0:DH, j * P:(j + 1) * P],
                                qpT[pr][0:DH, q0:q0 + QW],
                                start=True, stop=True)
                            nc.tensor.matmul(
                                sp[:, QW:2 * QW],
                                kpT[pr][DH:P, j * P:(j + 1) * P],
                                qpT[pr][DH:P, q0:q0 + QW],
                                start=True, stop=True)
                            nc.scalar.activation(
                                es[:, jj * 2 * QW:(jj + 1) * 2 * QW], sp[:],
                                AF.Exp, scale=SCALE)
                        for hloc, pv_ in ((0, pvA), (1, pvB)):
                            lhs = vp_dr[jp].rearrange(
                                "p (jj h e) -> p jj h e", jj=2, e=VPAD)[
                                :, :, 2 * pr + hloc, 0:DH + 1]
                            rhs = es.rearrange("p (jj he) -> p jj he", jj=2)[
                                :, :, hloc * QW:(hloc + 1) * QW]
                            nc.tensor.matmul(
                                pv_[:], lhs, rhs,
                                start=(jp == 0), stop=(jp == KB // 2 - 1),
                                perf_mode=DR)
                    # normalize + send both heads' q-chunk
                    for hloc, pv_ in ((0, pvA), (1, pvB)):
                        h = 2 * pr + hloc
                        srow = normp.tile([1, QW], f32, tag="srow", name="srow")
                        nc.vector.tensor_copy(srow[:], pv_[DH:DH + 1, :])
                        rec = normp.tile([1, QW], f32, tag="rec", name="rec")
                        nc.vector.reciprocal_approx_fast(rec[:], srow[:])
                        recB = normp.tile([DH, QW], f32, tag="recB", name="recB")
                        nc.gpsimd.partition_broadcast(recB[:], rec[:])
                        ctxn = normp.tile([DH, QW], bf16, tag="ctxn", name="ctxn")
                        nc.vector.tensor_mul(ctxn[:], pv_[0:DH, :], recB[:])
                        if DEBUG and h == 0:
                            nc.sync.dma_start(dbg_ctx[:, q0:q0 + QW], ctxn[:])
                        # receiver sums slot pairs (j, j+4); zero the copy
                        # belonging to the other batch (zcol is 1/0 per rank)
                        ctx0 = normp.tile([DH, QW], bf16, tag="ctx0", name="ctx0")
                        nc.vector.tensor_scalar_mul(ctx0[:], ctxn[:],
                                                    zcol_t[0:DH, 0:1])
                        ctx1 = normp.tile([DH, QW], bf16, tag="ctx1", name="ctx1")
                        nc.vector.tensor_scalar_mul(ctx1[:], ctxn[:],
                                                    zcol_t[0:DH, 1:2])
                        nc.sync.dma_start(a2a_ins[pr][qc, hloc], ctx0[:])
                        nc.sync.dma_start(a2a_ins[pr][qc + 4, hloc], ctx1[:])
                    # gather pair 0's exchange once it has certainly landed;
                    # the wait hint keeps the scheduler from hoisting these
                    # sem-blocked DMAs ahead of this pair's normalize work
                    if pr == 1 and qc == 3:
                        with tc.tile_wait_until(0.25):
                            build_ga(0)
                # fire this pair's exchange once all q-chunks are sent
                nc.gpsimd.collective_compute(
                    "AllToAll", mybir.AluOpType.bypass,
                    replica_groups=[list(range(NCORES))],
                    ins=[a2a_ins[pr][:]], outs=[a2a_outs[pr][:]])

        # ---- out-proj on my 512-token slice ----
        # Two passes over ib groups: the m=0 half (heads 0/1, available after
        # pair 0's AllToAll) accumulates for ALL token tiles first, so the PE
        # stays busy while pair 1's AllToAll is still in flight; the m=1 half
        # + bias finishes each tile afterwards.
        with tc.tile_pool(name="opsum", bufs=1, space="PSUM") as opsum, \
             tc.tile_pool(name="outsb", bufs=2) as outsb:
            build_ga(1)
            pots = [opsum.tile([P, E], f32, tag=f"pot{tm}", name=f"pot{tm}")
                    for tm in range(TS // P)]
            for m in range(2):
                for tm in range(TS // P):
                    pot = pots[tm]
                    for n, gp in enumerate(range(G)):
                        ib = gp * 2 + m
                        for oc in range(E // QW):
                            nc.tensor.matmul(
                                pot[:, oc * QW:(oc + 1) * QW],
                                ga[ib][:, tm * P:(tm + 1) * P],
                                wo_r[:, ib * E + oc * QW:ib * E + oc * QW + QW],
                                start=(m == 0 and n == 0), stop=False)
                    if m == 1:
                        for oc in range(E // QW):
                            nc.tensor.matmul(
                                pot[:, oc * QW:(oc + 1) * QW],
                                ones_r[0:1, 0:P],
                                bo_r[0:1, oc * QW:(oc + 1) * QW],
                                start=False, stop=True)
                        ot = outsb.tile([P, E], f32)
                        nc.scalar.activation(ot[:], pot[:], AF.Copy)
                        nc.sync.dma_start(out[tm * P:(tm + 1) * P, :], ot[:])

    nc.compile()
    return nc


def _get_nc():
    if 'nc' not in _cache:
        _cache['nc'] = _build()
    return _cache['nc']


def kernel(q, k, v, mask, Wq, bq, Wk, bk, Wv, bv, Wo, bo):
    from concourse.bass_utils import run_bass_kernel_spmd
    import ml_dtypes

    nc = _get_nc()
    f32 = np.float32
    bft = ml_dtypes.bfloat16

    def _cvt(x):
        return np.ascontiguousarray(np.asarray(x, f32)).astype(bft)

    qT = [_cvt(np.asarray(q, f32)[b].T) for b in range(B)]
    kT = [_cvt(np.asarray(k, f32)[b].T) for b in range(B)]
    vT = [_cvt(np.asarray(v, f32)[b].T) for b in range(B)]
    WqT = _cvt(np.asarray(Wq, f32).T)
    WkT = _cvt(np.asarray(Wk, f32).T)
    WvT = _cvt(np.asarray(Wv, f32).T)
    WoT = _cvt(np.asarray(Wo, f32).T)
    bqf = np.asarray(bq, f32)
    bkf = np.asarray(bk, f32)
    bvf = np.asarray(bv, f32)
    bof = _cvt(np.asarray(bo, f32))
    onesr = np.ones((1, P), bft)
    maskf = (np.asarray(mask) != 0).astype(f32)  # [B, S]

    in_maps = []
    for r in range(NCORES):
        b, g = r // G, r % G
        cols = slice(g * EG, (g + 1) * EG)
        m_pb = np.ascontiguousarray(maskf[b].reshape(KB, P).T)       # [128,16]
        m_rep = np.ascontiguousarray(np.repeat(m_pb, G, axis=1))     # [128,64]
        # bias columns [P, 6]: q_m0, q_m1, k_m0, k_m1, v_m0, v_m1
        bc = np.stack([bqf[cols].reshape(2, P)[0], bqf[cols].reshape(2, P)[1],
                       bkf[cols].reshape(2, P)[0], bkf[cols].reshape(2, P)[1],
                       bvf[cols].reshape(2, P)[0], bvf[cols].reshape(2, P)[1]],
                      axis=1).astype(f32)
        zc = np.zeros((P, 2), f32)
        zc[:, b] = 1.0
        in_maps.append({
            "xqT": qT[b], "xkT": kT[b], "xvT": vT[b],
            "wqT": np.ascontiguousarray(WqT[:, cols]),
            "wkT": np.ascontiguousarray(WkT[:, cols]),
            "wvT": np.ascontiguousarray(WvT[:, cols]),
            "woT": WoT,
            "bcols": np.ascontiguousarray(bc),
            "zcols": zc,
            "bov": bof[None, :],
            "onesv": onesr,
            "mask_pb": m_pb, "maskrep": m_rep,
        })

    res = run_bass_kernel_spmd(nc, in_maps, core_ids=list(range(NCORES)),
                               **_cache.get('run_kwargs', {}))
    _cache['last_results'] = res

    full = np.empty((B, S, E), f32)
    for r in range(NCORES):
        b, g = r // G, r % G
        full[b, g * TS:(g + 1) * TS, :] = res.results[r]["out"]
    return full
